# revision 8
# baseline (speedup 1.0000x reference)
# Trainium2 Bass kernel for CausalSelfAttention (B=2, T=2048, C=1024, H=16, D=64)
# with periodic mask: causal AND (key_col % 4 != 3).
#
# Sharding (8 NeuronCores): core c = (b, g) with b = c//4 (batch), g = c%4
# (head group of 4 heads). Each core computes QKV for its 4 heads, attention,
# and a partial output projection y_heads @ Wp[rows]. Host sums the 4 partials
# per batch and adds bp (tensor-parallel reduce).
#
# Key device-side choices:
#  - x arrives pre-transposed per batch (xT = x[b].T, [C, T]) so the
#    contraction dim C sits on SBUF partitions for all QKV matmuls.
#  - The periodic mask is exploited as compaction: keys at t%4==3 are never
#    attended, so K^T and V are only computed at the 1536 kept positions.
#  - Scores are produced transposed (S^T[tk_kept, tq]) so softmax-normalized
#    probabilities feed the AV matmul directly as the moving operand, and the
#    attention output comes out as y^T[d, tq], which is exactly the stationary
#    operand layout for the output projection.
#  - Softmax row sums: V tiles carry a 64-wide all-ones block, so each AV
#    accumulation produces [64 x y ; 64 x replicated row-sums] in PSUM; a DVE
#    reciprocal + multiply normalizes during the PSUM->SBUF copy.
#  - All matmuls run as float32r (full PE rate at N>=256, fp32 storage).

import numpy as np

B, T, C, H, D = 2, 2048, 1024, 16, 64
HG = 4          # heads per core
CG = HG * D     # = 256 columns of C per core
TK = (T // 4) * 3   # 1536 kept key positions
NTK = TK // 128     # 12 kept-key chunks of 128
SCALE = 1.0 / 8.0   # 1/sqrt(D)

_CACHE = {}


def _split_multi_waits(nc, mybir):
    # The pinned walrus here encodes at most 1 sync-wait per instruction
    # (2 for EventSemaphore). Hoist excess waits onto standalone NoOps that
    # precede the instruction on the same engine.
    f = nc.m.functions[0]
    n = 0
    for b in f.blocks:
        insts = list(b.instructions)
        out = []
        changed = False
        for inst in insts:
            si = inst.sync_info
            if si is not None:
                waits = list(si.on_wait)
                cap = 2 if isinstance(inst, mybir.InstEventSemaphore) else 1
                if len(waits) > cap:
                    for w in waits[cap:]:
                        out.append(mybir.InstNoOp(
                            name=f"{inst.name}-ws{n}", engine=inst.engine,
                            ins=[], outs=[],
                            sync_info=mybir.SyncInfo(on_wait=[w], on_update=[])))
                        n += 1
                    inst.sync_info = mybir.SyncInfo(
                        on_wait=waits[:cap], on_update=list(si.on_update))
                    changed = True
            out.append(inst)
        if changed:
            b.instructions = out
    return n


def _build_bass():
    import concourse.bass as bass
    import concourse.tile as tile
    import concourse.mybir as mybir
    from contextlib import ExitStack

    f32 = mybir.dt.float32
    f32r = mybir.dt.float32r
    r = lambda ap: ap   # operands are declared float32r natively

    nc = bass.Bass("TRN2", debug=False, num_devices=8)

    xt_d = nc.dram_tensor("xt", [C, T], f32r, kind="ExternalInput").ap()
    wq_d = nc.dram_tensor("wq", [C, CG], f32r, kind="ExternalInput").ap()
    wk_d = nc.dram_tensor("wk", [C, CG], f32r, kind="ExternalInput").ap()
    wv_d = nc.dram_tensor("wv", [C, CG], f32r, kind="ExternalInput").ap()
    wp_d = nc.dram_tensor("wp", [CG, C], f32r, kind="ExternalInput").ap()
    bq_d = nc.dram_tensor("bq2", [128, 2], f32, kind="ExternalInput").ap()
    bk_d = nc.dram_tensor("bk2", [128, 2], f32, kind="ExternalInput").ap()
    bvb_d = nc.dram_tensor("bvb", [128, HG, D], f32, kind="ExternalInput").ap()
    cm_d = nc.dram_tensor("cmask", [128, 3, 512], f32, kind="ExternalInput").ap()
    gs_d = nc.dram_tensor("gsel", [128, 6, 128], f32r, kind="ExternalInput").ap()
    on_d = nc.dram_tensor("vones", [128, HG, D], f32r, kind="ExternalInput").ap()
    out_d = nc.dram_tensor("out", [T, C], f32, kind="ExternalOutput").ap()

    with tile.TileContext(nc) as tc, \
         tc.tile_pool(name="persist", bufs=1) as persist:
        # ---------- persistent SBUF ----------
        qt = [persist.tile([128, T], f32r, name=f"qt{m}", tag=f"qt{m}") for m in range(2)]
        kt = [persist.tile([128, TK], f32r, name=f"kt{m}", tag=f"kt{m}") for m in range(2)]
        vsb = [persist.tile([128, HG, 2 * D], f32r, name=f"v{i}", tag=f"v{i}")
               for i in range(NTK)]
        yt = [persist.tile([128, T], f32r, name=f"yt{m}", tag=f"yt{m}") for m in range(2)]
        cmask = persist.tile([128, 3, 512], f32, name="cmask", tag="cmask")
        bqs = persist.tile([128, 2], f32, name="bqs", tag="bqs")
        bks = persist.tile([128, 2], f32, name="bks", tag="bks")
        bvb = persist.tile([128, HG, D], f32, name="bvb", tag="bvb")
        bvf = bvb[:].rearrange("p h d -> p (h d)")
        wp_t = persist.tile([128, 2, C], f32r, name="wp_t", tag="wp_t")

        nc.sync.dma_start(cmask[:], cm_d[:])
        nc.sync.dma_start(bqs[:], bq_d[:])
        nc.sync.dma_start(bks[:], bk_d[:])
        nc.sync.dma_start(bvb[:], bvb_d[:])
        nc.sync.dma_start(wp_t[:], wp_d.rearrange("(k p) n -> p k n", p=128))

        # ones blocks of V tiles (cols D..2D of each head block) — DMA'd
        # from host (memset can't write float32r)
        for i in range(NTK):
            nc.sync.dma_start(vsb[i][:, :, D:2 * D], on_d[:])

        # ---------- phase 1: QKV projections ----------
        with tc.tile_pool(name="p1", bufs=1) as p1, \
             tc.tile_pool(name="ps1", space="PSUM", bufs=4) as ps1:
            wq_t = p1.tile([128, 8, CG], f32r, tag="wq_t")
            wk_t = p1.tile([128, 8, CG], f32r, tag="wk_t")
            wv_t = p1.tile([128, 8, CG], f32r, tag="wv_t")
            nc.sync.dma_start(wq_t[:], wq_d.rearrange("(k p) n -> p k n", p=128))
            nc.sync.dma_start(wk_t[:], wk_d.rearrange("(k p) n -> p k n", p=128))
            nc.sync.dma_start(wv_t[:], wv_d.rearrange("(k p) n -> p k n", p=128))
            xt = []
            for k in range(8):
                xk = p1.tile([128, T], f32r, name=f"xt{k}", tag=f"xt{k}")
                nc.sync.dma_start(xk[:], xt_d[128 * k:128 * (k + 1), :])
                xt.append(xk)

            Ident = mybir.ActivationFunctionType.Identity

            # Q^T [2*128, T] and K^T [2*128, TK]
            for m in range(2):
                for j in range(4):
                    pq = ps1.tile([128, 512], f32, tag="pqk")
                    for k in range(8):
                        nc.tensor.matmul(
                            pq[:], r(wq_t[:, k, 128 * m:128 * (m + 1)]),
                            r(xt[k][:, 512 * j:512 * (j + 1)]),
                            start=(k == 0), stop=(k == 7))
                    nc.scalar.activation(qt[m][:, 512 * j:512 * (j + 1)], pq[:],
                                         Ident, bias=bqs[:, m:m + 1], scale=1.0)
                    pk = ps1.tile([128, 512], f32, tag="pqk")
                    for k in range(8):
                        nc.tensor.matmul(pk[:], r(wk_t[:, k, 128 * m:128 * (m + 1)]),
                                         r(xt[k][:, 512 * j:512 * (j + 1)]),
                                         start=(k == 0), stop=(k == 7))
                    # compact to kept key columns (drop t%4==3) during the copy
                    pkc = pk[:].rearrange("p (a b) -> p a b", b=4)[:, :, 0:3]
                    nc.scalar.activation(kt[m][:, 384 * j:384 * (j + 1)], pkc,
                                         Ident, bias=bks[:, m:m + 1], scale=1.0)

            # V at all positions first (V_full), then gather the kept rows
            # through the PE with 0/1 selection matmuls (rows = partitions,
            # so an AP gather can't do it).
            gsel = p1.tile([128, 6, 128], f32r, tag="gsel")
            nc.sync.dma_start(gsel[:], gs_d[:])
            vfull = []
            for mp in range(16):
                pv = ps1.tile([128, CG], f32, tag="pv", bufs=2)
                for k in range(8):
                    nc.tensor.matmul(pv[:], r(xt[k][:, 128 * mp:128 * (mp + 1)]),
                                     r(wv_t[:, k, :]), start=(k == 0), stop=(k == 7))
                vf = p1.tile([128, CG], f32r, name=f"vf{mp}", tag=f"vf{mp}")
                nc.vector.scalar_tensor_tensor(
                    out=vf[:], in0=pv[:], scalar=1.0, in1=bvf[:],
                    op0=mybir.AluOpType.bypass, op1=mybir.AluOpType.add)
                vfull.append(vf)
            for i in range(NTK):
                kq, s = divmod(i, 3)
                a0 = 4 * kq + s  # first original chunk this kept chunk draws from
                pvk = ps1.tile([128, CG], f32, tag="pvk", bufs=2)
                nc.tensor.matmul(pvk[:], r(gsel[:, 2 * s, :]), r(vfull[a0][:]),
                                 start=True, stop=False)
                nc.tensor.matmul(pvk[:], r(gsel[:, 2 * s + 1, :]), r(vfull[a0 + 1][:]),
                                 start=False, stop=True)
                nc.vector.tensor_copy(
                    vsb[i][:, :, 0:D],
                    pvk[:].rearrange("p (h d) -> p h d", d=D))

        # ---------- phase 2: attention ----------
        Exp = mybir.ActivationFunctionType.Exp
        MULT = mybir.AluOpType.mult
        with tc.tile_pool(name="p2", bufs=1) as p2, \
             tc.tile_pool(name="ps_s", space="PSUM", bufs=2) as ps_s, \
             tc.tile_pool(name="ps_y", space="PSUM", bufs=2) as ps_y:
            for h in range(HG):
                mq = h // 2
                pb = 64 * (h % 2)   # partition base of this head in qt/kt
                for j in range(4):
                    py = ps_y.tile([128, 512], f32, tag="py")
                    ngrp = j + 1
                    for gidx in range(ngrp):
                        ps3 = ps_s.tile([128, 3, 512], f32, tag="ps3")
                        pt3 = p2.tile([128, 3, 512], f32r, tag="pt3", bufs=3)
                        for q in range(3):
                            i = 3 * gidx + q
                            nc.tensor.matmul(
                                ps3[:, q, :],
                                r(kt[mq][pb:pb + 64, 128 * i:128 * (i + 1)]),
                                r(qt[mq][pb:pb + 64, 512 * j:512 * (j + 1)]),
                                start=True, stop=True)
                        nc.scalar.activation(pt3[:], ps3[:], Exp, bias=0.0,
                                             scale=SCALE)
                        if gidx == ngrp - 1:  # boundary group: causal masks
                            for u, w in ((0, 192), (1, 384), (2, 512)):
                                nc.vector.tensor_tensor(
                                    pt3[:, u, 0:w], pt3[:, u, 0:w],
                                    cmask[:, u, 0:w], op=MULT)
                        for q in range(3):
                            i = 3 * gidx + q
                            nc.tensor.matmul(
                                py[:], r(vsb[i][:, h, :]), r(pt3[:, q, :]),
                                start=(i == 0), stop=(i == 3 * ngrp - 1))
                    rec = p2.tile([64, 512], f32, tag="rec", bufs=2)
                    nc.vector.reciprocal(rec[:], py[64:128, :])
                    nc.vector.tensor_tensor(
                        yt[mq][pb:pb + 64, 512 * j:512 * (j + 1)],
                        py[0:64, :], rec[:], op=MULT)

        # ---------- phase 3: output projection ----------
        with tc.tile_pool(name="p3", bufs=3) as p3, \
             tc.tile_pool(name="ps_o", space="PSUM", bufs=4) as ps_o:
            for m in range(16):
                stage = p3.tile([128, C], f32, tag="stage")
                for n in range(2):
                    po = ps_o.tile([128, 512], f32, tag="po")
                    for k2 in range(2):
                        nc.tensor.matmul(
                            po[:], r(yt[k2][:, 128 * m:128 * (m + 1)]),
                            r(wp_t[:, k2, 512 * n:512 * (n + 1)]),
                            start=(k2 == 0), stop=(k2 == 1))
                    if n == 0:
                        nc.vector.tensor_copy(stage[:, 0:512], po[:])
                    else:
                        nc.scalar.copy(stage[:, 512:1024], po[:])
                nc.sync.dma_start(out_d[128 * m:128 * (m + 1), :], stage[:])

    _split_multi_waits(nc, mybir)
    return nc


def _get_nc():
    if "nc" not in _CACHE:
        _CACHE["nc"] = _build_bass()
    return _CACHE["nc"]


def _host_maps(inputs):
    x = np.asarray(inputs["x"], np.float32)
    Wq = np.asarray(inputs["Wq"], np.float32)
    Wk = np.asarray(inputs["Wk"], np.float32)
    Wv = np.asarray(inputs["Wv"], np.float32)
    Wp = np.asarray(inputs["Wp"], np.float32)
    bq = np.asarray(inputs["bq"], np.float32)
    bk = np.asarray(inputs["bk"], np.float32)
    bv = np.asarray(inputs["bv"], np.float32)

    # causal masks in compacted key coordinates: 3 boundary chunks
    p = np.arange(128)
    f = np.arange(512)
    cm = np.zeros((128, 3, 512), np.float32)
    for u in range(3):
        q = 128 * u + p
        g = (q // 3) * 4 + (q % 3)
        cm[:, u, :] = (f[None, :] >= g[:, None]).astype(np.float32)

    # V row-gather selection matrices: kept chunk i = 3k+s draws rows from
    # original chunks 4k+s and 4k+s+1; G[s][side][p, m] = 1 iff kept row m
    # maps to row p of that original chunk.
    gs = np.zeros((128, 6, 128), np.float32)
    for s in range(3):
        for m in range(128):
            orr = ((128 * s + m) // 3) * 4 + (128 * s + m) % 3
            side = 0 if orr < 128 * (s + 1) else 1
            gs[orr - 128 * (s + side), 2 * s + side, m] = 1.0

    xts = [np.ascontiguousarray(x[b].T) for b in range(B)]
    maps = []
    for c in range(8):
        b, g = c // 4, c % 4
        sl = slice(CG * g, CG * (g + 1))
        maps.append({
            "xt": xts[b],
            "wq": np.ascontiguousarray(Wq[:, sl]),
            "wk": np.ascontiguousarray(Wk[:, sl]),
            "wv": np.ascontiguousarray(Wv[:, sl]),
            "wp": np.ascontiguousarray(Wp[sl, :]),
            "bq2": np.ascontiguousarray(bq[sl].reshape(2, 128).T),
            "bk2": np.ascontiguousarray(bk[sl].reshape(2, 128).T),
            "bvb": np.ascontiguousarray(
                np.broadcast_to(bv[sl].reshape(HG, D), (128, HG, D))),
            "cmask": cm,
            "gsel": gs,
            "vones": np.ones((128, HG, D), np.float32),
        })
    return maps


def _combine(results, inputs):
    bp = np.asarray(inputs["bp"], np.float32)
    out = np.zeros((B, T, C), np.float32)
    for c in range(8):
        out[c // 4] += results[c]["out"]
    out += bp[None, None, :]
    return out


def _run(inputs, profile_dir=None, trace_cores=None):
    nc = _get_nc()
    maps = _host_maps(inputs)
    from concourse.bass_utils import run_bass_kernel_spmd
    if profile_dir is not None:
        import types, sys
        from trn_agent_boot.trn_boot import _ntff_profile_via_ctypes
        hook = _ntff_profile_via_ctypes("/opt/axon/libaxon_pjrt.so")
        with hook(profile_dir, trace_cores or [0]):
            res = run_bass_kernel_spmd(nc, maps, core_ids=list(range(8)))
    else:
        res = run_bass_kernel_spmd(nc, maps, core_ids=list(range(8)))
    return _combine(res.results, inputs)


def kernel(**inputs):
    return _run(inputs)
